# revision 12
# baseline (speedup 1.0000x reference)
"""DFT spectrogram (nn_DftSpectrogram) Bass kernel for 8 Trainium2 NeuronCores.

Pure data parallel: 32 batch items -> 4 per core. Per item (T=96512 samples):
  - 601 frames of 512 taps (stride 160) are loaded as 5 overlapping blocks of
    128 frames in [frame, tap] layout (contiguous 2KB rows -> efficient DMA)
  - PE-transposed to [tap, frame] and matmul'd against the DFT bases
    (only k<256 is needed; W is fed pre-transposed from the host)
  - log-magnitude + per-frame mean/std normalization over the 256 freqs is
    done in [frame, k] layout where the reduction is a native free-dim reduce
  - result is PE-transposed back to [k, frame] and DMA'd out

sqrt(var) is computed as exp(0.5*ln(var)) (ln and exp share one ACT table set,
avoiding a ~2.7us table switch to the sqrt set) and refined with one Heron
step on the vector engine; eps compensation keeps the algebra exact:
  (fft - mean)/(std + 1e-7) == (g - mean_g)/(std_g + 2*ln(10)*1e-7)
for fft = g * 0.5/ln(10).
"""
from contextlib import ExitStack

import numpy as np

import concourse.bass as bass
import concourse.tile as tile
from concourse import bacc, mybir
from concourse.bass_utils import run_bass_kernel_spmd

N_CORES = 8
B_FULL = 32
C_FULL = 1
T = 96512
NFFT = 512
KOUT = 256          # only lower half of the spectrum is kept
SHIFT = 160
F = (T - NFFT) // SHIFT + 1  # 601
BPC = B_FULL // N_CORES      # 4 items per core
EPS = 1e-7
CEPS = float(2.0 * np.log(10.0) * 1e-7)
F0S = (0, 128, 256, 384, 473)  # frame-block starts; last block overlaps by 39
FP32 = mybir.dt.float32

# matmul input dtype: float32 is exact (4 cycles/row on PE); float32r runs at
# full rate for N>=256 with slightly reduced precision.
MM_DT = mybir.dt.float32


def _build(ctx: ExitStack, tc: "tile.TileContext", xh, wrh, wih, idh, outh,
           mm_dt, reps: int):
    nc = tc.nc
    AP = bass.AP

    consts = ctx.enter_context(tc.tile_pool(name="consts", bufs=1))
    fpool = ctx.enter_context(tc.tile_pool(name="frames", bufs=2))
    ftpool = ctx.enter_context(tc.tile_pool(name="framesT", bufs=3))
    glpool = ctx.enter_context(tc.tile_pool(name="gl", bufs=2))
    mpool = ctx.enter_context(tc.tile_pool(name="mag", bufs=2))
    spool = ctx.enter_context(tc.tile_pool(name="stats", bufs=2))
    gnpool = ctx.enter_context(tc.tile_pool(name="gnorm", bufs=2))
    opool = ctx.enter_context(tc.tile_pool(name="outsb", bufs=2))
    ptrp = ctx.enter_context(tc.tile_pool(name="ptr", bufs=2, space="PSUM"))
    prip = ctx.enter_context(tc.tile_pool(name="pri", bufs=2, space="PSUM"))
    pop = ctx.enter_context(tc.tile_pool(name="po", bufs=2, space="PSUM"))

    # persistent constants
    wr_sb = consts.tile([128, 4 * KOUT], FP32, tag="wr_sb")
    wi_sb = consts.tile([128, 4 * KOUT], FP32, tag="wi_sb")
    ident = consts.tile([128, 128], FP32, tag="ident")
    epsb = consts.tile([128, 1], FP32, tag="epsb")
    nc.vector.memset(epsb[:], EPS)
    nc.sync.dma_start(wr_sb[:].rearrange("p (c k) -> p c k", k=KOUT),
                      wrh.ap().rearrange("(c p) k -> p c k", p=128))
    nc.sync.dma_start(wi_sb[:].rearrange("p (c k) -> p c k", k=KOUT),
                      wih.ap().rearrange("(c p) k -> p c k", p=128))
    nc.sync.dma_start(ident[:], idh.ap())

    def body():
        for b in range(BPC):
            xoff = b * T
            # frame blocks 0..3 in one DMA: ftile[p, t, n] = x[b, 160*(128t+p) + n]
            ftile = fpool.tile([128, 4 * NFFT], FP32, tag="ftile")
            src = AP(xh, xoff, [[SHIFT, 128], [SHIFT * 128, 4], [1, NFFT]])
            nc.sync.dma_start(ftile[:].rearrange("p (t n) -> p t n", n=NFFT), src)
            # frame block 4 (frames 473..600): exactly reaches x[96511]
            ftile2 = fpool.tile([128, NFFT], FP32, tag="ftile2")
            src2 = AP(xh, xoff + SHIFT * F0S[4], [[SHIFT, 128], [1, NFFT]])
            nc.sync.dma_start(ftile2[:], src2)

            gl = glpool.tile([128, 5 * KOUT], FP32, tag="gl")
            gsum = spool.tile([128, 5], FP32, tag="gsum")
            ssum = spool.tile([128, 5], FP32, tag="ssum")

            for fb in range(5):
                src_f = ftile[:, fb * NFFT:(fb + 1) * NFFT] if fb < 4 else ftile2[:]
                # transpose [128f, 512n] -> 4x [128n, 128f] packed in one PSUM bank
                ptr = ptrp.tile([128, NFFT], FP32, tag="ptr")
                for c in range(4):
                    nc.tensor.matmul(ptr[:, c * 128:(c + 1) * 128],
                                     src_f[:, c * 128:(c + 1) * 128], ident[:],
                                     is_transpose=True,
                                     start=(c == 0), stop=(c == 3))
                ft_sb = ftpool.tile([128, NFFT], FP32, tag="ft_sb")
                nc.scalar.copy(ft_sb[:], ptr[:])

                # DFT: out[f, k] = sum_n frames[f, n] * W[n, k], k < 256
                pri = prip.tile([128, 2 * KOUT], FP32, tag="pri")
                for c in range(4):
                    lhsT = ft_sb[:, c * 128:(c + 1) * 128].bitcast(mm_dt)
                    nc.tensor.matmul(pri[:, 0:KOUT], lhsT,
                                     wr_sb[:, c * KOUT:(c + 1) * KOUT].bitcast(mm_dt),
                                     start=(c == 0), stop=False)
                    nc.tensor.matmul(pri[:, KOUT:2 * KOUT], lhsT,
                                     wi_sb[:, c * KOUT:(c + 1) * KOUT].bitcast(mm_dt),
                                     start=False, stop=(c == 3))

                rr = mpool.tile([128, KOUT], FP32, tag="rr")
                ii = mpool.tile([128, KOUT], FP32, tag="ii")
                nc.scalar.square(rr[:], pri[:, 0:KOUT])
                nc.scalar.square(ii[:], pri[:, KOUT:2 * KOUT])
                msum = mpool.tile([128, KOUT], FP32, tag="msum")
                nc.vector.tensor_add(msum[:], rr[:], ii[:])
                gls = gl[:, fb * KOUT:(fb + 1) * KOUT]
                # g = ln(r^2 + i^2 + eps); accum_out gives sum_k g for free
                nc.scalar.activation(gls, msum[:], mybir.ActivationFunctionType.Ln,
                                     bias=epsb[:], accum_out=gsum[:, fb:fb + 1])
                scratch = mpool.tile([128, KOUT], FP32, tag="scratch")
                nc.scalar.activation(scratch[:], gls,
                                     mybir.ActivationFunctionType.Square,
                                     accum_out=ssum[:, fb:fb + 1])

            # stats: mean, var = E[g^2]-mean^2, rden = 1/(std + ceps)
            mean = spool.tile([128, 5], FP32, tag="mean")
            nc.vector.tensor_scalar_mul(mean[:], gsum[:], 1.0 / KOUT)
            msq = spool.tile([128, 5], FP32, tag="msq")
            nc.vector.tensor_mul(msq[:], mean[:], mean[:])
            e2 = spool.tile([128, 5], FP32, tag="e2")
            nc.vector.tensor_scalar_mul(e2[:], ssum[:], 1.0 / KOUT)
            var = spool.tile([128, 5], FP32, tag="var")
            nc.vector.tensor_sub(var[:], e2[:], msq[:])
            # sqrt(var) = exp(0.5*ln(var)) (same ACT table set as Ln), then one
            # Heron step s1 = 0.5*(s0 + var/s0) for fp32-level accuracy
            lnv = spool.tile([128, 5], FP32, tag="lnv")
            nc.scalar.activation(lnv[:], var[:], mybir.ActivationFunctionType.Ln)
            s0 = spool.tile([128, 5], FP32, tag="s0")
            nc.scalar.activation(s0[:], lnv[:], mybir.ActivationFunctionType.Exp,
                                 scale=0.5)
            rs0 = spool.tile([128, 5], FP32, tag="rs0")
            nc.vector.reciprocal(rs0[:], s0[:])
            q = spool.tile([128, 5], FP32, tag="q")
            nc.vector.tensor_mul(q[:], var[:], rs0[:])
            s1 = spool.tile([128, 5], FP32, tag="s1")
            nc.vector.tensor_add(s1[:], s0[:], q[:])
            u = spool.tile([128, 5], FP32, tag="u")
            nc.vector.tensor_scalar(u[:], s1[:], 0.5, CEPS,
                                    op0=mybir.AluOpType.mult,
                                    op1=mybir.AluOpType.add)
            rden = spool.tile([128, 5], FP32, tag="rden")
            nc.vector.reciprocal(rden[:], u[:])

            out0 = opool.tile([128, F], FP32, tag="out0")
            out1 = opool.tile([128, F], FP32, tag="out1")
            for fb in range(5):
                gls = gl[:, fb * KOUT:(fb + 1) * KOUT]
                gn = gnpool.tile([128, KOUT], FP32, tag="gn")
                nc.vector.tensor_scalar(gn[:], gls,
                                        mean[:, fb:fb + 1], rden[:, fb:fb + 1],
                                        op0=mybir.AluOpType.subtract,
                                        op1=mybir.AluOpType.mult)
                po = pop.tile([128, KOUT], FP32, tag="po")
                nc.tensor.matmul(po[:, 0:128], gn[:, 0:128], ident[:],
                                 is_transpose=True, start=True, stop=False)
                nc.tensor.matmul(po[:, 128:256], gn[:, 128:256], ident[:],
                                 is_transpose=True, start=False, stop=True)
                f0 = F0S[fb]
                if fb < 4:
                    nc.scalar.copy(out0[:, f0:f0 + 128], po[:, 0:128])
                    nc.scalar.copy(out1[:, f0:f0 + 128], po[:, 128:256])
                else:
                    # frames 473..511 were already written by block 3
                    nc.scalar.copy(out0[:, 512:601], po[:, 39:128])
                    nc.scalar.copy(out1[:, 512:601], po[:, 128 + 39:256])

            nc.sync.dma_start(outh.ap()[b, 0:128, :], out0[:])
            nc.sync.dma_start(outh.ap()[b, 128:256, :], out1[:])

    if reps == 1:
        body()
    else:
        with tc.For_i(0, reps, 1):
            body()


def build_nc(mm_dt=MM_DT, reps: int = 1):
    nc = bacc.Bacc("TRN2", target_bir_lowering=False, debug=False)
    xh = nc.dram_tensor("x", [BPC, T], FP32, kind="ExternalInput")
    wrh = nc.dram_tensor("wr", [NFFT, KOUT], FP32, kind="ExternalInput")
    wih = nc.dram_tensor("wi", [NFFT, KOUT], FP32, kind="ExternalInput")
    idh = nc.dram_tensor("ident", [128, 128], FP32, kind="ExternalInput")
    outh = nc.dram_tensor("out", [BPC, KOUT, F], FP32, kind="ExternalOutput")
    with tile.TileContext(nc) as tc, ExitStack() as ctx:
        _build(ctx, tc, xh, wrh, wih, idh, outh, mm_dt, reps)
    nc.compile()
    return nc


def make_in_maps(x, W_real, W_imag):
    xs = np.asarray(x, dtype=np.float32).reshape(B_FULL, T)
    wr_dev = np.ascontiguousarray(np.asarray(W_real, np.float32)[:KOUT, :].T)
    wi_dev = np.ascontiguousarray(np.asarray(W_imag, np.float32)[:KOUT, :].T)
    ident = np.eye(128, dtype=np.float32)
    return [
        {"x": np.ascontiguousarray(xs[i * BPC:(i + 1) * BPC]),
         "wr": wr_dev, "wi": wi_dev, "ident": ident}
        for i in range(N_CORES)
    ]


_NC_CACHE = {}


def kernel(x, W_real, W_imag):
    key = (str(MM_DT), 1)
    if key not in _NC_CACHE:
        _NC_CACHE[key] = build_nc(MM_DT, 1)
    nc = _NC_CACHE[key]
    in_maps = make_in_maps(x, W_real, W_imag)
    res = run_bass_kernel_spmd(nc, in_maps, core_ids=list(range(N_CORES)))
    out = np.concatenate([r["out"] for r in res.results], axis=0)
    return out.reshape(B_FULL, C_FULL, KOUT, F).astype(np.float32)
